# revision 1
# baseline (speedup 1.0000x reference)
"""Trainium2 Bass kernel for nn_AllSparkModule (retrieval_knn).

Sharding: pure data-parallel - one batch sample per NeuronCore (8 samples,
8 cores). Cores 0-3 run the labeled (cross-attention vs kv_queue) branch,
cores 4-7 the unlabeled (channel self-attention) branch, selected at runtime
with a partition_id branch inside one SPMD program. No collectives.

Key reassociations vs the straightforward pipeline (x = sample [256, 16384]):
  A: emb = gelu(w_in @ x) fp16 SBUF-resident; emb^T tiles via PE transpose
     (fp8 for labeled / fp16 for unlabeled); Graw = emb kv^T accumulates in
     fp8 DoubleRow (labeled), G2 = emb emb^T in fp16 (unlabeled).
  B: scores from Graw/G2 + rank-1 bias terms (tiny GEMMs); inorm+softmax.
     The phase-C/D InstanceNorm stats are computed ANALYTICALLY here:
       z = wo attn kv + emb  (labeled)  -> E[z], E[z^2] from Graw, Ckv
       z = P emb + b0 1^T    (unlabeled) -> from G2
     so w_out, the inorm affine, and wo fold into single matrices:
       labeled:   y = gelu(W1 @ kv + W2 @ emb + c2),  W1 = w_out s2 wo attn
       unlabeled: y = gelu(W  @ emb + c2),            W  = w_out s2 P
  C+D: one streaming pass; labeled W1@kv runs in fp8 DoubleRow on a
     KSC-scaled kv copy, accumulated separately and combined on GpSimd.

y is emitted fp16 (cast to fp32 on host); kv streams are fp8.
"""
import os
import numpy as np
import ml_dtypes

import concourse.bass as bass
import concourse.mybir as mybir
import concourse.tile as tile
from concourse.bass_utils import run_bass_kernel_spmd
from concourse.masks import make_identity
from concourse.vector_clock import ScopedClock

F32 = mybir.dt.float32
F32R = mybir.dt.float32r
BF16 = mybir.dt.bfloat16
F16 = mybir.dt.float16
FP8 = mybir.dt.float8e4
AF = mybir.ActivationFunctionType
ALU = mybir.AluOpType
AX = mybir.AxisListType
DR = mybir.MatmulPerfMode.DoubleRow

P = 128          # partitions
C = 256          # channels
NCH = 2          # channel chunks
N = 16384        # feature length
TILE = 512       # free-dim tile
NSUB = TILE // P  # 128-col subtiles per tile
NT_FULL = N // TILE
J_L = 512        # labeled keys (nc*ec)
J_U = 256        # unlabeled keys (ec)
SCALE = float(N) ** -0.5
EPS = 1e-5
KSC = 256.0      # kv fp8 pre-scale
W1S = 64.0       # W1T fp8 pre-scale
W1DQ = 1.0 / (KSC * W1S)  # folded into the final gelu's scale

BUFS_IO = int(os.environ.get("KERNEL_BUFS_IO", "3"))

# ---------------------------------------------------------------------------
# Workarounds for the pinned walrus: max ONE sync-wait per instruction.


class _TC(tile.TileContext):
    def _drain_and_barrier(self, tick_clock, wait_clock):
        drain_inst = self.nc.sync.drain()
        wait_clock.add_sem_waits(
            drain_inst.ins, ScopedClock({None: tick_clock.global_clock})
        )
        si = drain_inst.ins.sync_info
        if si is not None and si.on_wait and len(si.on_wait) > 1:
            waits = list(si.on_wait)
            drain_inst.ins.sync_info = mybir.SyncInfo(
                on_wait=waits[:1], on_update=list(si.on_update))
            for w in waits[1:]:
                d2 = self.nc.sync.drain()
                d2.ins.sync_info = mybir.SyncInfo(on_wait=[w], on_update=[])
        self.nc.all_engine_barrier()
        assert self.sems is not None
        popped = self.nc._tile_sem_poison_stack.pop()
        assert popped is self._sem_poison
        self.nc.clear_and_free_semaphores(list(self.sems.allocated().values()))
        self.nc.all_engine_barrier()


def _split_sync_waits(nc, max_waits: int = 1):
    """Rewrite sync_info for the pinned walrus: at most one wait per
    instruction, and no instruction (other than barrier EventSemaphores)
    that both waits on and updates the same semaphore. Extra/conflicting
    waits are hoisted onto NOPs inserted just before, on the same engine
    stream, which preserves ordering."""
    ctr = 0
    for fn in nc.m.functions:
        for bb in fn.blocks:
            insts = bb.instructions
            new = []
            changed = False
            for inst in insts:
                si = getattr(inst, "sync_info", None)
                waits = list(si.on_wait) if si is not None and si.on_wait else []
                upd = list(si.on_update) if si is not None and si.on_update else []
                conflict = False
                iname = type(inst).__name__
                if waits and upd and iname != "InstEventSemaphore":
                    upd_ids = {u.id for u in upd}
                    conflict = any(w.id in upd_ids for w in waits)
                    if not conflict and iname != "InstDMACopy":
                        imm_upd = any(
                            str(getattr(u, "update_mode", "")).endswith("imm")
                            for u in upd)
                        conflict = (imm_upd
                                    or inst.engine == mybir.EngineType.Pool)
                if len(waits) > max_waits or conflict:
                    keep = [] if conflict else waits[-max_waits:]
                    extras = waits if conflict else waits[:-max_waits]
                    for s in range(0, len(extras), max_waits):
                        chunk = extras[s:s + max_waits]
                        nop = mybir.InstNoOp(
                            name=f"waitsplit_{ctr}", ins=[], outs=[])
                        ctr += 1
                        nop.engine = inst.engine
                        nop.sync_info = mybir.SyncInfo(
                            on_wait=list(chunk), on_update=[])
                        new.append(nop)
                    inst.sync_info = mybir.SyncInfo(
                        on_wait=list(keep), on_update=list(upd))
                    changed = True
                new.append(inst)
            if changed:
                bb.instructions = new
    return ctr


# ---------------------------------------------------------------------------


def _build(nt: int, branch: str | None = None):
    """Build the SPMD program processing the first nt 512-column tiles."""
    nc = bass.Bass()

    x_in = nc.declare_dram_parameter("x", [C, N], F16, isOutput=False)
    kvt_in = nc.declare_dram_parameter("kvt", [N, J_L], FP8, isOutput=False)
    kv_in = nc.declare_dram_parameter("kv", [J_L, N], FP8, isOutput=False)
    ckv_in = nc.declare_dram_parameter("ckv", [J_L, J_L], F16, isOutput=False)
    w_inT = nc.declare_dram_parameter("w_inT", [C, C], F16, isOutput=False)
    woT16_in = nc.declare_dram_parameter("woT16", [C, C], F16, isOutput=False)
    wonat_in = nc.declare_dram_parameter("wo_nat16", [C, C], F16,
                                         isOutput=False)
    wvnat_in = nc.declare_dram_parameter("wv_nat16", [C, C], F16,
                                         isOutput=False)
    w_outT16_in = nc.declare_dram_parameter("w_outT16", [C, C], F16,
                                            isOutput=False)
    wqT = nc.declare_dram_parameter("wqT", [C, C], F32R, isOutput=False)
    wkT = nc.declare_dram_parameter("wkT", [C, C], F32R, isOutput=False)
    wvT = nc.declare_dram_parameter("wvT", [C, C], F32R, isOutput=False)
    affn = nc.declare_dram_parameter("affn", [4, C], F32, isOutput=False)
    kvsum_in = nc.declare_dram_parameter("kvsum", [P, J_L], F32,
                                         isOutput=False)
    kvsT_in = nc.declare_dram_parameter("kvsumT", [J_L, 2], F16,
                                        isOutput=False)
    y_out = nc.declare_dram_parameter("y", [C, N], F16, isOutput=True)
    debug = bool(int(os.environ.get("KERNEL_DEBUG", "0")))
    debug_mid = bool(int(os.environ.get("KERNEL_DEBUG_MID", "0")))
    if debug:
        dbg_out = nc.declare_dram_parameter("dbg", [P, 8, C], F32,
                                            isOutput=True)

    # [p, ch, n] views of the channel-major DRAM tensors
    x_r = x_in[:, :].rearrange("(c p) n -> p c n", p=P)
    y_r = y_out[:, :].rearrange("(c p) n -> p c n", p=P)
    # [t, p, s, j] view of kv^T rows grouped per 512-col tile
    kvt_r = kvt_in[:, :].rearrange("(t s p) j -> t p s j", p=P, s=NSUB)
    # [t, p, s, n] view of kv with the 512 key rows split into 4 chunks
    kv_r = kv_in[:, :].rearrange("(s p) (t n) -> t p s n", p=P, n=TILE)
    ckv_r = ckv_in[:, :].rearrange("(s p) j -> p s j", p=P)
    kvsT_r = kvsT_in[:, :].rearrange("(s p) c -> p s c", p=P)
    affn_r = affn[:, :].rearrange("a (c p) -> p a c", p=P)

    inv_n = 1.0 / float(nt * TILE)

    with _TC(nc) as tc:
        pid = nc.partition_id()

        import contextlib
        stack = contextlib.ExitStack()
        with stack:
            singles = stack.enter_context(tc.tile_pool(name="singles", bufs=1))
            res = stack.enter_context(tc.tile_pool(name="res", bufs=1))

            # ---- persistent tiles -------------------------------------
            emb_res = res.tile([P, NCH, N], F16)

            w_inT_s = singles.tile([P, NCH, C], F16)
            woT_s = singles.tile([P, NCH, C], F16)
            wonat_s = singles.tile([P, NCH, C], F16)
            wvnat_s = singles.tile([P, NCH, C], F16)
            w_outT_s = singles.tile([P, NCH, C], F16)
            wqT_s = singles.tile([P, NCH, C], F32R)
            wkT_s = singles.tile([P, NCH, C], F32R)
            wvT_s = singles.tile([P, NCH, C], F32R)
            for dst, src in ((w_inT_s, w_inT), (woT_s, woT16_in),
                             (wonat_s, wonat_in), (wvnat_s, wvnat_in),
                             (w_outT_s, w_outT16_in), (wqT_s, wqT),
                             (wkT_s, wkT), (wvT_s, wvT)):
                nc.sync.dma_start(
                    out=dst, in_=src[:, :].rearrange("(c p) o -> p c o", p=P))

            affn_s = singles.tile([P, 4, NCH], F32)
            nc.sync.dma_start(out=affn_s, in_=affn_r)

            ident16 = singles.tile([P, P], F16)
            make_identity(nc, ident16)
            ident_f = singles.tile([P, P], F32)
            make_identity(nc, ident_f)
            ones16 = singles.tile([P, P], F16)
            nc.vector.memset(ones16, 1.0)

            eps_t = singles.tile([P, 1], F32)
            nc.vector.memset(eps_t, EPS)

            esum_t = singles.tile([P, NCH, nt], F32)
            esq_t = singles.tile([P, nt], F32)
            st_e0 = singles.tile([P, nt, 6], F32)
            s1_t = singles.tile([P, NCH], F32)
            b1_t = singles.tile([P, NCH], F32)
            me_t = singles.tile([P, NCH], F32)
            ve_t = singles.tile([P, NCH], F32)
            s2_t = singles.tile([P, NCH], F32)
            b2_t = singles.tile([P, NCH], F32)
            c2_t = singles.tile([P, NCH], F32)
            mz_t = singles.tile([P, NCH], F32)

            # me_t and ve_t must be filled by branch-specific code first;
            # computes s1 = gamma*rstd, b1 = beta - mean*s1
            def _fold_stats():
                sd = singles.tile([P, NCH], F32, tag="sd1")
                tmp = singles.tile([P, NCH], F32, tag="tmp1")
                for ch in range(NCH):
                    nc.scalar.activation(
                        out=sd[:, ch:ch + 1], in_=ve_t[:, ch:ch + 1],
                        func=AF.Sqrt, bias=eps_t)
                    nc.vector.reciprocal(
                        out=sd[:, ch:ch + 1], in_=sd[:, ch:ch + 1])
                    nc.vector.tensor_mul(
                        s1_t[:, ch:ch + 1], sd[:, ch:ch + 1],
                        affn_s[:, 0, ch:ch + 1])
                    nc.vector.tensor_mul(
                        tmp[:, ch:ch + 1], me_t[:, ch:ch + 1],
                        s1_t[:, ch:ch + 1])
                    nc.vector.tensor_sub(
                        b1_t[:, ch:ch + 1], affn_s[:, 1, ch:ch + 1],
                        tmp[:, ch:ch + 1])

            # ez2 [P, NCH] f32 (E[z^2]); mz_t set beforehand -> s2, b2, c2.
            # b0_t is the column-constant part of z (z = ... + b0 1^T), so
            # c2 = w_out @ (s2*b0 + b2); pass None when z has no such term.
            def _fold_s2_c2(smp, psB, ez2, b0_t=None):
                vz = smp.tile([P, NCH], F32, tag="vz")
                tmp = smp.tile([P, NCH], F32, tag="tmpz")
                nc.vector.tensor_mul(tmp, mz_t, mz_t)
                nc.vector.tensor_sub(vz, ez2, tmp)
                for ch in range(NCH):
                    nc.scalar.activation(
                        out=vz[:, ch:ch + 1], in_=vz[:, ch:ch + 1],
                        func=AF.Sqrt, bias=eps_t)
                    nc.vector.reciprocal(
                        out=vz[:, ch:ch + 1], in_=vz[:, ch:ch + 1])
                    nc.vector.tensor_mul(
                        s2_t[:, ch:ch + 1], vz[:, ch:ch + 1],
                        affn_s[:, 2, ch:ch + 1])
                    nc.vector.tensor_mul(
                        tmp[:, ch:ch + 1], mz_t[:, ch:ch + 1],
                        s2_t[:, ch:ch + 1])
                    nc.vector.tensor_sub(
                        b2_t[:, ch:ch + 1], affn_s[:, 3, ch:ch + 1],
                        tmp[:, ch:ch + 1])
                # W2T = diag(s2) w_outT  (fp16)
                w2t = smp.tile([P, NCH, C], F16, tag="w2t")
                for cc in range(NCH):
                    nc.vector.tensor_scalar(
                        out=w2t[:, cc, :], in0=w_outT_s[:, cc, :],
                        scalar1=s2_t[:, cc:cc + 1], scalar2=None,
                        op0=ALU.mult)
                # c2 = w_out @ (s2*b0 + b2)
                cb = smp.tile([P, NCH], F32, tag="cb")
                nc.vector.tensor_copy(cb, b2_t)
                if b0_t is not None:
                    nc.vector.tensor_mul(tmp, s2_t, b0_t)
                    nc.vector.tensor_add(cb, cb, tmp)
                b2r = smp.tile([P, NCH, 2], F16, tag="b2r")
                nc.vector.memset(b2r, 0.0)
                for ch in range(NCH):
                    nc.vector.tensor_copy(b2r[:, ch, 0:1], cb[:, ch:ch + 1])
                for oc in range(NCH):
                    c2_ps = psB.tile([P, 2], F32, tag="tiny")
                    for cc in range(NCH):
                        nc.tensor.matmul(
                            c2_ps, w_outT_s[:, cc, bass.ts(oc, P)],
                            b2r[:, cc, :],
                            start=(cc == 0), stop=(cc == NCH - 1))
                    nc.vector.tensor_copy(c2_t[:, oc:oc + 1], c2_ps[:, 0:1])
                return w2t

            # scores [P, NCH, j] -> attn_sm [P, NCH, j] f16 (softmaxed)
            # and attnT [P, j//P, C] f16 (transposed)
            def _softmax_transpose(scores_ps, j_dim, pool, pst):
                attn_pre = pool.tile([P, NCH, j_dim], F32, tag="attnpre")
                attn_sm = pool.tile([P, NCH, j_dim], F16, tag="attnsm")
                ms = pool.tile([P, NCH, 6], F32, tag="sm_stats")
                mv = pool.tile([P, NCH, 2], F32, tag="sm_mv")
                sd = pool.tile([P, NCH], F32, tag="sm_sd")
                mx = pool.tile([P, NCH], F32, tag="sm_mx")
                sm = pool.tile([P, NCH], F32, tag="sm_sum")
                for ch in range(NCH):
                    nc.vector.bn_stats(
                        out=ms[:, ch, :], in_=scores_ps[ch])
                    nc.vector.bn_aggr(out=mv[:, ch, :], in_=ms[:, ch, :])
                    nc.scalar.activation(
                        out=sd[:, ch:ch + 1], in_=mv[:, ch, 1:2],
                        func=AF.Sqrt, bias=eps_t)
                    nc.vector.reciprocal(
                        out=sd[:, ch:ch + 1], in_=sd[:, ch:ch + 1])
                    nc.vector.tensor_scalar(
                        out=attn_pre[:, ch, :], in0=scores_ps[ch],
                        scalar1=mv[:, ch, 0:1], scalar2=sd[:, ch:ch + 1],
                        op0=ALU.subtract, op1=ALU.mult)
                    nc.vector.tensor_reduce(
                        out=mx[:, ch:ch + 1], in_=attn_pre[:, ch, :],
                        axis=AX.X, op=ALU.max, negate=True)
                    nc.scalar.activation(
                        out=attn_pre[:, ch, :], in_=attn_pre[:, ch, :],
                        func=AF.Exp, bias=mx[:, ch:ch + 1],
                        accum_out=sm[:, ch:ch + 1])
                    nc.vector.reciprocal(
                        out=sm[:, ch:ch + 1], in_=sm[:, ch:ch + 1])
                    nc.vector.tensor_scalar_mul(
                        out=attn_sm[:, ch, :], in0=attn_pre[:, ch, :],
                        scalar1=sm[:, ch:ch + 1])
                n_j = j_dim // P
                attnT = pool.tile([P, n_j, C], F16, tag="attnT")
                for ch in range(NCH):
                    for s in range(n_j):
                        tp_ps = pst.tile([P, P], F16, tag="tp")
                        nc.tensor.transpose(
                            tp_ps, attn_sm[:, ch, bass.ts(s, P)], ident16)
                        nc.vector.tensor_copy(
                            attnT[:, s, bass.ts(ch, P)], tp_ps)
                return attn_sm, attnT

            import contextlib as _ctxlib

            def _if_lab():
                return (tc.If(pid < 4) if branch is None
                        else _ctxlib.nullcontext())

            def _else(cmp):
                return (cmp.Else() if branch is None
                        else _ctxlib.nullcontext())

            lab_cm = _if_lab()
            with lab_cm as cmp:
              if branch in (None, "lab"):
                # ======== LABELED: cross-attention vs kv queue =========
                with tc.tile_pool(name="smL", bufs=1) as smp:
                    ckv_s = smp.tile([P, NSUB, J_L], F16, tag="ckv")
                    nc.sync.dma_start(out=ckv_s, in_=ckv_r)
                    kvs_t = smp.tile([P, J_L], F32, tag="kvs")
                    nc.sync.dma_start(out=kvs_t, in_=kvsum_in[:, :])
                    kvsT_s = smp.tile([P, NSUB, 2], F16, tag="kvsT")
                    nc.sync.dma_start(out=kvsT_s, in_=kvsT_r)

                    # ---- phase A: map_in + stats + emb^T + Graw -------
                    with tc.tile_pool(name="psGL", bufs=1,
                                      space="PSUM") as psG:
                        gr_ps = [psG.tile([P, J_L], F32, tag=f"gr{a}",
                                          name=f"grl{a}")
                                 for a in range(NCH)]
                        with tc.tile_pool(name="phAL", bufs=2) as ioA, \
                             tc.tile_pool(name="psAL", bufs=2,
                                          space="PSUM") as psA, \
                             tc.tile_pool(name="psTL", bufs=2,
                                          space="PSUM") as psT:
                            for t in range(nt):
                                ts = bass.ts(t, TILE)
                                x_t = ioA.tile([P, NCH, TILE], F16, tag="x")
                                nc.sync.dma_start(out=x_t, in_=x_r[:, :, ts])
                                kvt_t = ioA.tile([P, NSUB, J_L], FP8,
                                                 tag="kvt")
                                nc.sync.dma_start(out=kvt_t, in_=kvt_r[t])
                                e_ps = psA.tile([P, NCH, TILE], F32,
                                                tag="eps")
                                for oc in range(NCH):
                                    for cc in range(NCH):
                                        nc.tensor.matmul(
                                            e_ps[:, oc, :],
                                            w_inT_s[:, cc, bass.ts(oc, P)],
                                            x_t[:, cc, :],
                                            start=(cc == 0),
                                            stop=(cc == NCH - 1))
                                for oc in range(NCH):
                                    nc.scalar.activation(
                                        out=emb_res[:, oc, ts],
                                        in_=e_ps[:, oc, :], func=AF.Gelu,
                                        accum_out=esum_t[:, oc, t:t + 1])
                                # variance inputs: ch0 on DVE, ch1 on ACT
                                nc.vector.bn_stats(
                                    out=st_e0[:, t, :],
                                    in_=emb_res[:, 0, ts])
                                sqs = ioA.tile([P, TILE], F16, tag="sqs")
                                nc.scalar.activation(
                                    out=sqs, in_=emb_res[:, 1, ts],
                                    func=AF.Square,
                                    accum_out=esq_t[:, t:t + 1])
                                et8 = ioA.tile([P, NSUB, C], FP8, tag="et")
                                for cc in range(NCH):
                                    tp4 = psT.tile([P, NSUB, P], F16,
                                                   tag="tp")
                                    for s in range(NSUB):
                                        nc.tensor.transpose(
                                            tp4[:, s, :],
                                            emb_res[:, cc,
                                                    bass.ts(t * NSUB + s, P)],
                                            ident16)
                                    nc.vector.tensor_copy(
                                        et8[:, :, bass.ts(cc, P)], tp4)
                                for sp in range(NSUB // 2):
                                    for a in range(NCH):
                                        nc.tensor.matmul(
                                            gr_ps[a],
                                            et8[:, 2 * sp:2 * sp + 2,
                                                bass.ts(a, P)],
                                            kvt_t[:, 2 * sp:2 * sp + 2, :],
                                            start=(t == 0 and sp == 0),
                                            stop=(t == nt - 1
                                                  and sp == NSUB // 2 - 1),
                                            perf_mode=DR)
                        # me from gelu accums; ve: ch0 bn_aggr, ch1 esq
                        tmpf = smp.tile([P, NCH], F32, tag="tmpf")
                        for ch in range(NCH):
                            nc.vector.tensor_reduce(
                                out=tmpf[:, ch:ch + 1], in_=esum_t[:, ch, :],
                                axis=AX.X, op=ALU.add)
                        nc.scalar.mul(out=me_t, in_=tmpf, mul=inv_n)
                        mv0 = smp.tile([P, 2], F32, tag="mv0")
                        nc.vector.bn_aggr(out=mv0, in_=st_e0)
                        nc.vector.tensor_copy(ve_t[:, 0:1], mv0[:, 1:2])
                        nc.vector.tensor_reduce(
                            out=ve_t[:, 1:2], in_=esq_t, axis=AX.X,
                            op=ALU.add)
                        nc.scalar.mul(out=ve_t[:, 1:2], in_=ve_t[:, 1:2],
                                      mul=inv_n)
                        nc.vector.tensor_mul(tmpf, me_t, me_t)
                        nc.vector.tensor_sub(
                            ve_t[:, 1:2], ve_t[:, 1:2], tmpf[:, 1:2])
                        _fold_stats()
                        gr_sb = smp.tile([P, NCH, J_L], F32R, tag="grsb")
                        for a in range(NCH):
                            nc.scalar.copy(out=gr_sb[:, a, :], in_=gr_ps[a])

                    # ---- phase B: scores, softmax, analytic z-stats ---
                    with tc.tile_pool(name="psBL", bufs=2,
                                      space="PSUM") as psB:
                        b1r = smp.tile([P, NCH, 2], F32R, tag="b1r")
                        padl = smp.tile([P, NCH, 2], F32, tag="padl")
                        nc.vector.memset(padl, 0.0)
                        nc.vector.tensor_copy(b1r, padl)
                        for ch in range(NCH):
                            nc.vector.tensor_copy(
                                b1r[:, ch, 0:1], b1_t[:, ch:ch + 1])
                        s1s = smp.tile([P, NCH], F32, tag="s1s")
                        nc.scalar.mul(out=s1s, in_=s1_t, mul=SCALE / KSC)
                        wqf = smp.tile([P, NCH, C], F32R, tag="wqf")
                        for cc in range(NCH):
                            nc.vector.tensor_scalar(
                                out=wqf[:, cc, :], in0=wqT_s[:, cc, :],
                                scalar1=s1s[:, cc:cc + 1], scalar2=None,
                                op0=ALU.mult)
                        bqs = smp.tile([P, NCH], F32, tag="bqs")
                        for ic in range(NCH):
                            bq_ps = psB.tile([P, 2], F32, tag="tiny")
                            for cc in range(NCH):
                                nc.tensor.matmul(
                                    bq_ps, wqT_s[:, cc, bass.ts(ic, P)],
                                    b1r[:, cc, :],
                                    start=(cc == 0), stop=(cc == NCH - 1))
                            nc.scalar.mul(out=bqs[:, ic:ic + 1],
                                          in_=bq_ps[:, 0:1], mul=SCALE)
                        scores_c = smp.tile([P, NCH, J_L], F32,
                                            tag="scoresc")
                        for ic in range(NCH):
                            sc_ps = psB.tile([P, J_L], F32, tag="big")
                            for cc in range(NCH):
                                nc.tensor.matmul(
                                    sc_ps, wqf[:, cc, bass.ts(ic, P)],
                                    gr_sb[:, cc, :],
                                    start=(cc == 0), stop=(cc == NCH - 1))
                            nc.vector.scalar_tensor_tensor(
                                out=scores_c[:, ic, :], in0=kvs_t,
                                scalar=bqs[:, ic:ic + 1], in1=sc_ps,
                                op0=ALU.mult, op1=ALU.add)
                        attn_sm, attnT = _softmax_transpose(
                            [scores_c[:, 0, :], scores_c[:, 1, :]],
                            J_L, smp, psB)

                        # A = wo attn (fp32 sbuf) and AT (fp16)
                        A_sb = smp.tile([P, NCH, J_L], F32, tag="Asb")
                        for oc in range(NCH):
                            a_ps = psB.tile([P, J_L], F32, tag="big")
                            for qc in range(NCH):
                                nc.tensor.matmul(
                                    a_ps, woT_s[:, qc, bass.ts(oc, P)],
                                    attn_sm[:, qc, :],
                                    start=(qc == 0), stop=(qc == NCH - 1))
                            nc.vector.tensor_copy(A_sb[:, oc, :], a_ps)
                        AT_sb = smp.tile([P, NSUB, C], F16, tag="ATsb")
                        for jc in range(NSUB):
                            at_ps = psB.tile([P, C], F32, tag="med")
                            for qc in range(NCH):
                                nc.tensor.matmul(
                                    at_ps,
                                    attn_sm[:, qc, bass.ts(jc, P)],
                                    woT_s[:, qc, :],
                                    start=(qc == 0), stop=(qc == NCH - 1))
                            nc.vector.tensor_copy(AT_sb[:, jc, :], at_ps)

                        # diag1 = rowsum((A Ckv) o A); diag2 = rowsum(A o Gr)
                        dg = smp.tile([P, NCH, 2], F32, tag="dg")
                        dtmp = smp.tile([P, J_L], F32, tag="dtmp")
                        for ic in range(NCH):
                            t_ps = psB.tile([P, J_L], F32, tag="big")
                            for jc in range(NSUB):
                                nc.tensor.matmul(
                                    t_ps, AT_sb[:, jc, bass.ts(ic, P)],
                                    ckv_s[:, jc, :],
                                    start=(jc == 0), stop=(jc == NSUB - 1))
                            nc.vector.tensor_mul(dtmp, t_ps, A_sb[:, ic, :])
                            nc.vector.tensor_reduce(
                                out=dg[:, ic, 0:1], in_=dtmp, axis=AX.X,
                                op=ALU.add)
                            nc.vector.tensor_mul(
                                dtmp, A_sb[:, ic, :], gr_sb[:, ic, :])
                            nc.vector.tensor_reduce(
                                out=dg[:, ic, 1:2], in_=dtmp, axis=AX.X,
                                op=ALU.add)

                        # m_z = (wo attn kvsum)/N + m_e
                        v1f = smp.tile([P, NCH, 2], F16, tag="v1f")
                        for qc in range(NCH):
                            v1_ps = psB.tile([P, 2], F32, tag="tiny")
                            for jc in range(NSUB):
                                nc.tensor.matmul(
                                    v1_ps, attnT[:, jc, bass.ts(qc, P)],
                                    kvsT_s[:, jc, :],
                                    start=(jc == 0), stop=(jc == NSUB - 1))
                            nc.vector.tensor_copy(v1f[:, qc, :], v1_ps)
                        for oc in range(NCH):
                            v2_ps = psB.tile([P, 2], F32, tag="tiny")
                            for qc in range(NCH):
                                nc.tensor.matmul(
                                    v2_ps, woT_s[:, qc, bass.ts(oc, P)],
                                    v1f[:, qc, :],
                                    start=(qc == 0), stop=(qc == NCH - 1))
                            nc.vector.scalar_tensor_tensor(
                                out=mz_t[:, oc:oc + 1], in0=v2_ps[:, 0:1],
                                scalar=inv_n, in1=me_t[:, oc:oc + 1],
                                op0=ALU.mult, op1=ALU.add)

                        # E[z^2] = diag1/N + 2 diag2/(N*KSC) + v_e + m_e^2
                        ez2 = smp.tile([P, NCH], F32, tag="ez2")
                        etmp = smp.tile([P, NCH], F32, tag="etmp")
                        nc.vector.tensor_mul(etmp, me_t, me_t)
                        nc.vector.tensor_add(ez2, ve_t, etmp)
                        for ic in range(NCH):
                            nc.vector.scalar_tensor_tensor(
                                out=ez2[:, ic:ic + 1], in0=dg[:, ic, 0:1],
                                scalar=inv_n, in1=ez2[:, ic:ic + 1],
                                op0=ALU.mult, op1=ALU.add)
                            nc.vector.scalar_tensor_tensor(
                                out=ez2[:, ic:ic + 1], in0=dg[:, ic, 1:2],
                                scalar=2.0 * inv_n / KSC,
                                in1=ez2[:, ic:ic + 1],
                                op0=ALU.mult, op1=ALU.add)
                        w2t = _fold_s2_c2(smp, psB, ez2)

                        # M2 = wo^T diag(s2) w_out^T ; W1T8 = attn^T M2 * W1S
                        M2_sb = smp.tile([P, NCH, C], F16, tag="M2")
                        for qc in range(NCH):
                            m2_ps = psB.tile([P, C], F32, tag="med")
                            for ic in range(NCH):
                                nc.tensor.matmul(
                                    m2_ps, wonat_s[:, ic, bass.ts(qc, P)],
                                    w2t[:, ic, :],
                                    start=(ic == 0), stop=(ic == NCH - 1))
                            nc.vector.tensor_copy(M2_sb[:, qc, :], m2_ps)
                        w1t8 = smp.tile([P, NSUB, C], FP8, tag="w1t8")
                        for jc in range(NSUB):
                            w1_ps = psB.tile([P, C], F32, tag="med")
                            for qc in range(NCH):
                                nc.tensor.matmul(
                                    w1_ps, attn_sm[:, qc, bass.ts(jc, P)],
                                    M2_sb[:, qc, :],
                                    start=(qc == 0), stop=(qc == NCH - 1))
                            nc.scalar.activation(
                                out=w1t8[:, jc, :], in_=w1_ps,
                                func=AF.Identity, scale=W1S)
                        # emb-side weights pre-scaled to the kv-psum scale
                        w2ts = smp.tile([P, NCH, C], F16, tag="w2ts")
                        for cc in range(NCH):
                            nc.scalar.mul(out=w2ts[:, cc, :],
                                          in_=w2t[:, cc, :], mul=KSC * W1S)

                    # ---- phase C+D: y = gelu(W1 kv + W2 emb + c2) -----
                    with tc.tile_pool(name="phCL", bufs=BUFS_IO) as ioC, \
                         tc.tile_pool(name="psOL", bufs=3,
                                      space="PSUM") as psO:
                        for t in range(nt):
                            ts = bass.ts(t, TILE)
                            kv_t = ioC.tile([P, NSUB, TILE], FP8, tag="kv")
                            nc.sync.dma_start(out=kv_t, in_=kv_r[t])
                            o_ps = psO.tile([P, NCH, TILE], F32, tag="ops")
                            for oc in range(NCH):
                                for jp in range(NSUB // 2):
                                    nc.tensor.matmul(
                                        o_ps[:, oc, :],
                                        w1t8[:, 2 * jp:2 * jp + 2,
                                             bass.ts(oc, P)],
                                        kv_t[:, 2 * jp:2 * jp + 2, :],
                                        start=(jp == 0), stop=False,
                                        perf_mode=DR,
                                        skip_group_check=True)
                                for cc in range(NCH):
                                    nc.tensor.matmul(
                                        o_ps[:, oc, :],
                                        w2ts[:, cc, bass.ts(oc, P)],
                                        emb_res[:, cc, ts],
                                        start=False,
                                        stop=(cc == NCH - 1),
                                        skip_group_check=True)
                            y_t = ioC.tile([P, NCH, TILE], F16, tag="yt")
                            for oc in range(NCH):
                                nc.scalar.activation(
                                    out=y_t[:, oc, :], in_=o_ps[:, oc, :],
                                    func=AF.Gelu, scale=W1DQ,
                                    bias=c2_t[:, oc:oc + 1])
                            nc.sync.dma_start(out=y_r[:, :, ts], in_=y_t)

            with _else(cmp):
              if branch in (None, "unl"):
                # ======== UNLABELED: Gram-matrix self-attention ========
                ntot = float(nt * TILE)
                with tc.tile_pool(name="smU", bufs=1) as smp:
                    # ---- phase A: map_in + stats + emb^T + Gram -------
                    with tc.tile_pool(name="psGU", bufs=1,
                                      space="PSUM") as psG:
                        # symmetry: full a=0 rows + the [128:256]^2 block;
                        # B10 is reconstructed as B01^T afterwards
                        g2_ps0 = psG.tile([P, C], F32, tag="g20",
                                          name="g2u0")
                        g2_ps1 = psG.tile([P, P], F32, tag="g21",
                                          name="g2u1")
                        with tc.tile_pool(name="phAU", bufs=2) as ioA, \
                             tc.tile_pool(name="psAU", bufs=2,
                                          space="PSUM") as psA, \
                             tc.tile_pool(name="psTU", bufs=2,
                                          space="PSUM") as psT:
                            for t in range(nt):
                                ts = bass.ts(t, TILE)
                                x_t = ioA.tile([P, NCH, TILE], F16, tag="x")
                                nc.sync.dma_start(out=x_t, in_=x_r[:, :, ts])
                                e_ps = psA.tile([P, NCH, TILE], F32,
                                                tag="eps")
                                for oc in range(NCH):
                                    for cc in range(NCH):
                                        nc.tensor.matmul(
                                            e_ps[:, oc, :],
                                            w_inT_s[:, cc, bass.ts(oc, P)],
                                            x_t[:, cc, :],
                                            start=(cc == 0),
                                            stop=(cc == NCH - 1))
                                for oc in range(NCH):
                                    nc.scalar.activation(
                                        out=emb_res[:, oc, ts],
                                        in_=e_ps[:, oc, :], func=AF.Gelu,
                                        accum_out=esum_t[:, oc, t:t + 1])
                                et16 = ioA.tile([P, NSUB, C], F16, tag="et")
                                for cc in range(NCH):
                                    tp4 = psT.tile([P, NSUB, P], F16,
                                                   tag="tp")
                                    for s in range(NSUB):
                                        nc.tensor.transpose(
                                            tp4[:, s, :],
                                            emb_res[:, cc,
                                                    bass.ts(t * NSUB + s, P)],
                                            ident16)
                                    nc.vector.tensor_copy(
                                        et16[:, :, bass.ts(cc, P)], tp4)
                                for s in range(NSUB):
                                    st = (t == 0 and s == 0)
                                    sp = (t == nt - 1 and s == NSUB - 1)
                                    nc.tensor.matmul(
                                        g2_ps0, et16[:, s, 0:P],
                                        et16[:, s, :], start=st, stop=sp)
                                    nc.tensor.matmul(
                                        g2_ps1, et16[:, s, P:C],
                                        et16[:, s, P:C], start=st, stop=sp)
                        g2_sb = smp.tile([P, NCH, C], F32R, tag="g2sb")
                        g2_16 = smp.tile([P, NCH, C], F16, tag="g216")
                        nc.scalar.copy(out=g2_sb[:, 0, :], in_=g2_ps0)
                        nc.vector.tensor_copy(g2_16[:, 0, :], g2_ps0)
                        nc.scalar.copy(out=g2_sb[:, 1, P:C], in_=g2_ps1)
                        nc.vector.tensor_copy(g2_16[:, 1, P:C], g2_ps1)
                        # B10 = B01^T via PE transpose of the fp16 copy
                        with tc.tile_pool(name="psT2U", bufs=1,
                                          space="PSUM") as psT2:
                            tpb = psT2.tile([P, P], F16, tag="tpb")
                            nc.tensor.transpose(
                                tpb, g2_16[:, 0, P:C], ident16)
                            nc.vector.tensor_copy(g2_sb[:, 1, 0:P], tpb)
                            nc.vector.tensor_copy(g2_16[:, 1, 0:P], tpb)
                        # me from gelu accums; E[e^2] from the G2 diagonal
                        tmpf = smp.tile([P, NCH], F32, tag="tmpf")
                        for ch in range(NCH):
                            nc.vector.tensor_reduce(
                                out=tmpf[:, ch:ch + 1], in_=esum_t[:, ch, :],
                                axis=AX.X, op=ALU.add)
                        nc.scalar.mul(out=me_t, in_=tmpf, mul=inv_n)
                        dgt = smp.tile([P, P], F32, tag="dgt")
                        nc.vector.tensor_mul(dgt, g2_sb[:, 0, 0:P], ident_f)
                        nc.vector.tensor_reduce(
                            out=ve_t[:, 0:1], in_=dgt, axis=AX.X, op=ALU.add)
                        nc.vector.tensor_mul(dgt, g2_sb[:, 1, P:C], ident_f)
                        nc.vector.tensor_reduce(
                            out=ve_t[:, 1:2], in_=dgt, axis=AX.X, op=ALU.add)
                        nc.scalar.mul(out=ve_t, in_=ve_t, mul=inv_n)
                        nc.vector.tensor_mul(tmpf, me_t, me_t)
                        nc.vector.tensor_sub(ve_t, ve_t, tmpf)
                        _fold_stats()

                    with tc.tile_pool(name="psBU", bufs=2,
                                      space="PSUM") as psB:
                        # ---- scores via G2 + rank-1 terms (baseline) --
                        mvE = smp.tile([P, NCH, 2], F32, tag="mvE")
                        embsum_f = smp.tile([P, NCH], F32, tag="embsumf")
                        pad_f = smp.tile([P, NCH, 2], F32, tag="padf")
                        embsum_r = smp.tile([P, NCH, 2], F32R, tag="embsumr")
                        b1r = smp.tile([P, NCH, 2], F32R, tag="b1r")
                        nc.vector.memset(pad_f, 0.0)
                        for ch in range(NCH):
                            nc.scalar.mul(out=embsum_f[:, ch:ch + 1],
                                          in_=me_t[:, ch:ch + 1], mul=ntot)
                        nc.vector.tensor_copy(b1r, pad_f)
                        nc.vector.tensor_copy(embsum_r, pad_f)
                        for ch in range(NCH):
                            nc.vector.tensor_copy(
                                b1r[:, ch, 0:1], b1_t[:, ch:ch + 1])
                            nc.vector.tensor_copy(
                                embsum_r[:, ch, 0:1],
                                embsum_f[:, ch:ch + 1])
                        s1s = smp.tile([P, NCH], F32, tag="s1s")
                        nc.scalar.mul(out=s1s, in_=s1_t, mul=SCALE)
                        wqf = smp.tile([P, NCH, C], F32R, tag="wqf")
                        wkf = smp.tile([P, NCH, C], F32R, tag="wkf")
                        for cc in range(NCH):
                            nc.vector.tensor_scalar(
                                out=wqf[:, cc, :], in0=wqT_s[:, cc, :],
                                scalar1=s1s[:, cc:cc + 1], scalar2=None,
                                op0=ALU.mult)
                            nc.vector.tensor_scalar(
                                out=wkf[:, cc, :], in0=wkT_s[:, cc, :],
                                scalar1=s1_t[:, cc:cc + 1], scalar2=None,
                                op0=ALU.mult)
                        b1_rep = smp.tile([P, NCH, P], F32R, tag="b1rep")
                        es_rep = smp.tile([P, NCH, P], F32R, tag="esrep")
                        for cc in range(NCH):
                            nc.vector.tensor_scalar(
                                out=b1_rep[:, cc, :], in0=ones16,
                                scalar1=b1_t[:, cc:cc + 1], scalar2=None,
                                op0=ALU.mult)
                            nc.vector.tensor_scalar(
                                out=es_rep[:, cc, :], in0=ones16,
                                scalar1=embsum_f[:, cc:cc + 1], scalar2=None,
                                op0=ALU.mult)
                        bqs = smp.tile([P, NCH], F32, tag="bqs")
                        q0s = smp.tile([P, NCH], F32, tag="q0s")
                        bv = smp.tile([P, NCH], F32, tag="bv")
                        for ic in range(NCH):
                            bq_ps = psB.tile([P, 2], F32, tag="tiny")
                            for cc in range(NCH):
                                nc.tensor.matmul(
                                    bq_ps, wqT_s[:, cc, bass.ts(ic, P)],
                                    b1r[:, cc, :],
                                    start=(cc == 0), stop=(cc == NCH - 1))
                            nc.scalar.mul(out=bqs[:, ic:ic + 1],
                                          in_=bq_ps[:, 0:1], mul=SCALE)
                            q0_ps = psB.tile([P, 2], F32, tag="tiny")
                            for cc in range(NCH):
                                nc.tensor.matmul(
                                    q0_ps, wqf[:, cc, bass.ts(ic, P)],
                                    embsum_r[:, cc, :],
                                    start=(cc == 0), stop=(cc == NCH - 1))
                            nc.scalar.copy(out=q0s[:, ic:ic + 1],
                                           in_=q0_ps[:, 0:1])
                            bv_ps = psB.tile([P, 2], F32, tag="tiny")
                            for cc in range(NCH):
                                nc.tensor.matmul(
                                    bv_ps, wvT_s[:, cc, bass.ts(ic, P)],
                                    b1r[:, cc, :],
                                    start=(cc == 0), stop=(cc == NCH - 1))
                            nc.scalar.copy(out=bv[:, ic:ic + 1],
                                           in_=bv_ps[:, 0:1])
                        bk_full = smp.tile([P, J_U], F32, tag="bkfull")
                        r1_full = smp.tile([P, J_U], F32, tag="r1full")
                        bk_ps = psB.tile([P, J_U], F32, tag="med")
                        for cc in range(NCH):
                            nc.tensor.matmul(
                                bk_ps, b1_rep[:, cc, :], wkT_s[:, cc, :],
                                start=(cc == 0), stop=(cc == NCH - 1))
                        nc.scalar.copy(out=bk_full, in_=bk_ps)
                        ks_ps = psB.tile([P, J_U], F32, tag="med")
                        for cc in range(NCH):
                            nc.tensor.matmul(
                                ks_ps, es_rep[:, cc, :], wkf[:, cc, :],
                                start=(cc == 0), stop=(cc == NCH - 1))
                        nc.vector.tensor_scalar(
                            out=r1_full, in0=bk_full, scalar1=ntot,
                            scalar2=None, op0=ALU.mult)
                        nc.vector.tensor_add(r1_full, r1_full, ks_ps)
                        h_sb = smp.tile([P, NCH, C], F32R, tag="hsb")
                        for cb in range(NCH):
                            h_ps = psB.tile([P, C], F32, tag="med")
                            for a in range(NCH):
                                nc.tensor.matmul(
                                    h_ps, g2_sb[:, a, bass.ts(cb, P)],
                                    wkf[:, a, :],
                                    start=(a == 0), stop=(a == NCH - 1))
                            nc.scalar.copy(out=h_sb[:, cb, :], in_=h_ps)
                        scores_c = smp.tile([P, NCH, J_U], F32,
                                            tag="scoresc")
                        for ic in range(NCH):
                            sc_ps = psB.tile([P, J_U], F32, tag="med")
                            for cc in range(NCH):
                                nc.tensor.matmul(
                                    sc_ps, wqf[:, cc, bass.ts(ic, P)],
                                    h_sb[:, cc, :],
                                    start=(cc == 0), stop=(cc == NCH - 1))
                            nc.vector.scalar_tensor_tensor(
                                out=scores_c[:, ic, :], in0=r1_full,
                                scalar=bqs[:, ic:ic + 1], in1=sc_ps,
                                op0=ALU.mult, op1=ALU.add)
                            nc.vector.scalar_tensor_tensor(
                                out=scores_c[:, ic, :], in0=bk_full,
                                scalar=q0s[:, ic:ic + 1],
                                in1=scores_c[:, ic, :],
                                op0=ALU.mult, op1=ALU.add)
                        attn_sm, attnT = _softmax_transpose(
                            [scores_c[:, 0, :], scores_c[:, 1, :]],
                            J_U, smp, psB)

                        # ---- z-chain: P = wo attn wv diag(s1) + I -----
                        # b0 = wo attn bv
                        bvr = smp.tile([P, NCH, 2], F16, tag="bvr")
                        nc.vector.memset(bvr, 0.0)
                        for ch in range(NCH):
                            nc.vector.tensor_copy(
                                bvr[:, ch, 0:1], bv[:, ch:ch + 1])
                        v1f = smp.tile([P, NCH, 2], F16, tag="v1f")
                        for qc in range(NCH):
                            v1_ps = psB.tile([P, 2], F32, tag="tiny")
                            for jc in range(NCH):
                                nc.tensor.matmul(
                                    v1_ps, attnT[:, jc, bass.ts(qc, P)],
                                    bvr[:, jc, :],
                                    start=(jc == 0), stop=(jc == NCH - 1))
                            nc.vector.tensor_copy(v1f[:, qc, :], v1_ps)
                        b0_t = smp.tile([P, NCH], F32, tag="b0")
                        for oc in range(NCH):
                            b0_ps = psB.tile([P, 2], F32, tag="tiny")
                            for qc in range(NCH):
                                nc.tensor.matmul(
                                    b0_ps, woT_s[:, qc, bass.ts(oc, P)],
                                    v1f[:, qc, :],
                                    start=(qc == 0), stop=(qc == NCH - 1))
                            nc.vector.tensor_copy(
                                b0_t[:, oc:oc + 1], b0_ps[:, 0:1])

                        # s1 broadcast along free axis: s1bc[p, c] = s1[c]
                        # (replicate down free then PE-transpose the block)
                        s1rep = smp.tile([P, P], F16, tag="s1rep")
                        s1bc = smp.tile([P, C], F32, tag="s1bc")
                        for cc in range(NCH):
                            nc.vector.tensor_scalar(
                                out=s1rep, in0=ones16,
                                scalar1=s1_t[:, cc:cc + 1], scalar2=None,
                                op0=ALU.mult)
                            bc_ps = psB.tile([P, P], F16, tag="tp")
                            nc.tensor.transpose(bc_ps, s1rep, ident16)
                            nc.vector.tensor_copy(
                                s1bc[:, bass.ts(cc, P)], bc_ps)

                        # A0 = attn wv_nat, col-scaled by s1
                        A0s = smp.tile([P, NCH, C], F16, tag="A0s")
                        for qc in range(NCH):
                            a0_ps = psB.tile([P, C], F32, tag="med")
                            for jc in range(NCH):
                                nc.tensor.matmul(
                                    a0_ps, attnT[:, jc, bass.ts(qc, P)],
                                    wvnat_s[:, jc, :],
                                    start=(jc == 0), stop=(jc == NCH - 1))
                            nc.vector.tensor_mul(
                                A0s[:, qc, :], a0_ps, s1bc)
                        # P0 = wo A0s + I
                        P_sb = smp.tile([P, NCH, C], F16, tag="Psb")
                        for oc in range(NCH):
                            p_ps = psB.tile([P, C], F32, tag="med")
                            for qc in range(NCH):
                                nc.tensor.matmul(
                                    p_ps, woT_s[:, qc, bass.ts(oc, P)],
                                    A0s[:, qc, :],
                                    start=(qc == 0), stop=(qc == NCH - 1))
                            nc.vector.tensor_copy(P_sb[:, oc, :], p_ps)
                        for oc in range(NCH):
                            nc.vector.tensor_add(
                                P_sb[:, oc, bass.ts(oc, P)],
                                P_sb[:, oc, bass.ts(oc, P)], ident16)
                        # PT via PE transpose
                        PT_sb = smp.tile([P, NCH, C], F16, tag="PTsb")
                        for r in range(NCH):
                            for cb in range(NCH):
                                tp_ps = psB.tile([P, P], F16, tag="tp")
                                nc.tensor.transpose(
                                    tp_ps, P_sb[:, r, bass.ts(cb, P)],
                                    ident16)
                                nc.vector.tensor_copy(
                                    PT_sb[:, cb, bass.ts(r, P)], tp_ps)

                        # diag3 = rowsum((P G2) o P)
                        dg3 = smp.tile([P, NCH], F32, tag="dg3")
                        dtmp = smp.tile([P, C], F32, tag="dtmpU")
                        for ic in range(NCH):
                            t2_ps = psB.tile([P, C], F32, tag="med")
                            for mc in range(NCH):
                                nc.tensor.matmul(
                                    t2_ps, PT_sb[:, mc, bass.ts(ic, P)],
                                    g2_16[:, mc, :],
                                    start=(mc == 0), stop=(mc == NCH - 1))
                            nc.vector.tensor_mul(
                                dtmp, t2_ps, P_sb[:, ic, :])
                            nc.vector.tensor_reduce(
                                out=dg3[:, ic:ic + 1], in_=dtmp, axis=AX.X,
                                op=ALU.add)

                        # m_z = P m_e + b0
                        mer = smp.tile([P, NCH, 2], F16, tag="mer")
                        nc.vector.memset(mer, 0.0)
                        for ch in range(NCH):
                            nc.vector.tensor_copy(
                                mer[:, ch, 0:1], me_t[:, ch:ch + 1])
                        for ic in range(NCH):
                            mz_ps = psB.tile([P, 2], F32, tag="tiny")
                            for cc in range(NCH):
                                nc.tensor.matmul(
                                    mz_ps, PT_sb[:, cc, bass.ts(ic, P)],
                                    mer[:, cc, :],
                                    start=(cc == 0), stop=(cc == NCH - 1))
                            nc.vector.tensor_add(
                                mz_t[:, ic:ic + 1], mz_ps[:, 0:1],
                                b0_t[:, ic:ic + 1])

                        # E[z^2] = diag3/N + 2 b0 m_z - b0^2
                        ez2 = smp.tile([P, NCH], F32, tag="ez2")
                        etmp = smp.tile([P, NCH], F32, tag="etmp")
                        nc.vector.tensor_mul(etmp, b0_t, mz_t)
                        nc.vector.tensor_add(ez2, etmp, etmp)
                        nc.vector.tensor_mul(etmp, b0_t, b0_t)
                        nc.vector.tensor_sub(ez2, ez2, etmp)
                        for ic in range(NCH):
                            nc.vector.scalar_tensor_tensor(
                                out=ez2[:, ic:ic + 1], in0=dg3[:, ic:ic + 1],
                                scalar=inv_n, in1=ez2[:, ic:ic + 1],
                                op0=ALU.mult, op1=ALU.add)
                        w2t = _fold_s2_c2(smp, psB, ez2, b0_t=b0_t)

                        if os.environ.get("KERNEL_DBG1"):
                            scr1 = smp.tile([P, NCH, C], F32, tag="scr1")
                            for qc in range(NCH):
                                nc.vector.tensor_copy(
                                    scr1[:, qc, :], P_sb[:, qc, :])
                        if debug_mid:
                            dbg_sb = smp.tile([P, 8, C], F32, tag="dbg")
                            nc.vector.memset(dbg_sb, 0.0)
                            for qc in range(NCH):
                                nc.vector.tensor_copy(
                                    dbg_sb[:, 0 + qc, :], attn_sm[:, qc, :])
                                nc.vector.tensor_copy(
                                    dbg_sb[:, 2 + qc, :], P_sb[:, qc, :])
